# revision 40
# baseline (speedup 1.0000x reference)
"""AdaptiveGraphConv Trainium2 kernel, data-parallel over batch on 8 NeuronCores.

Reference computation (per full input):
  sim  = relu(E @ E^T)                               [N, N]
  d[n] = 1 + softmax(sim, axis=1)[n, n]              (diag gate)
  Ew   = einsum('nd,dcf->ncf', E, W)                 per-node weights
  eb   = E @ bias                                    per-node bias [N, F]
  y[b,t,n,f] = (d[n] * x[b,t,n,:]) @ Ew[n] + eb[n]

Device strategy per core (2 of 16 batches, R = 2*288 = 576 rows):
  - host supplies x transposed to node-major [N, C, R] in BF16 so each
    node-pair is a contiguous [128, 576] block (C on partitions = matmul
    contraction layout). BF16 on the wire halves the HBM traffic that bounds
    this kernel (~358 GB/s/core HBM limit); rel-err budget 2e-2 vs ~4.4e-3
    measured for bf16 x/Ew/y with f32 PSUM accumulation.
  - host packs NB=13 pairs per DMA group so each SBUF partition line is one
    contiguous 15 KB descriptor: with bf16's 1.15 KB/pair lines the DMA is
    descriptor-rate-bound (~5.7 ns/desc ~ 200 GB/s measured); grouping
    restores byte-bound streaming.
  - on-chip: compute d, fold it into E (E' = diag(d) @ E), build
    block-diagonal per-pair stationary weights Ew in BF16, then stream 8
    groups x 13 node-pairs: 2 bf16 matmuls per pair (row halves), bias-add
    from PSUM written in place over the x tile, alternating DVE / ACT per
    half so neither engine bottlenecks. Loads and stores ride ONE HWDGE ring
    in whole-pass bursts so HBM traffic stays unidirectional per burst
    (measured here: pure reads 700 GB/s/core, pure writes 676 GB/s/core,
    but any read/write alternation collapses the combined rate to ~300-450
    GB/s; separate concurrent load/store rings are worse still).
  - host un-permutes y^T shards back to [B, T, N, F] and widens to f32.
"""

import sys

sys.path.insert(0, "/opt/trn_rl_repo")

from contextlib import ExitStack

import numpy as np

N_CORES = 8
NODE = 207
NODE_P = 208  # padded to even node count
PAIRS = NODE_P // 2  # 104
EMB = 128
C = 64
F = 64
B = 16
T = 288
B_SH = B // N_CORES  # 2
R = B_SH * T  # 576 rows per core
RH = R // 2  # 288, matmul free-dim chunk
NB = 13  # pairs per DMA group (contiguous per-partition lines)
G = PAIRS // NB  # 8 groups

_CACHE = {}


def _build(
    repeat=1,
    bufs=None,
    ph=None,
    nb=NB,
    ring="sync",
    mode="normal",
    ncores=N_CORES,
    obufs=0,
):
    ng = PAIRS // nb
    if bufs is None:
        bufs = ng
    if ph is None:
        ph = ng
    import concourse.tile as tile
    from concourse import bacc, mybir

    f32 = mybir.dt.float32
    bf16 = mybir.dt.bfloat16
    AF = mybir.ActivationFunctionType
    ALU = mybir.AluOpType
    AX = mybir.AxisListType

    nc = bacc.Bacc("TRN2", target_bir_lowering=False, debug=False, num_devices=ncores)
    xt = nc.dram_tensor("xt", [ng * 128, nb * R], bf16, kind="ExternalInput").ap()
    emb = nc.dram_tensor("emb", [NODE_P, EMB], f32, kind="ExternalInput").ap()
    w = nc.dram_tensor("w", [EMB, F * C], bf16, kind="ExternalInput").ap()
    bias_d = nc.dram_tensor("bias", [EMB, F], f32, kind="ExternalInput").ap()
    eye = nc.dram_tensor("eye", [NODE_P, NODE], f32, kind="ExternalInput").ap()
    ident = nc.dram_tensor("ident", [128, 128], f32, kind="ExternalInput").ap()
    yt = nc.dram_tensor("yt", [ng * 128, nb * R], bf16, kind="ExternalOutput").ap()

    if mode in ("ld_only", "st_only", "ldst", "ldst2", "ldst_i"):
        # bandwidth diagnostics: raw DMA streams, no compute
        nt = 8 if mode in ("ldst2", "ldst_i") else bufs
        with tile.TileContext(nc) as tc, ExitStack() as ctx:
            dpool = ctx.enter_context(tc.tile_pool(name="dbw", bufs=1))
            tiles = [
                dpool.tile([128, nb * R], bf16, name=f"bw{i}", tag=f"bw{i}")
                for i in range(nt)
            ]
            for tle in tiles:
                nc.vector.memset(tle[:].bitcast(f32), 0.0)
            if mode == "ld_only":
                # the declared output must be written at least once
                nc.sync.dma_start(yt[0:128, :], tiles[0][:])
            # software-unrolled repeat (For_i needs every engine in the body)
            for _ in range(repeat):
                if mode == "ldst":
                    for k in range(ng):
                        nc.sync.dma_start(
                            tiles[k % nt][:], xt[k * 128 : (k + 1) * 128, :]
                        )
                    for k in range(ng):
                        nc.sync.dma_start(
                            yt[k * 128 : (k + 1) * 128, :], tiles[k % nt][:]
                        )
                elif mode == "ldst2":
                    # stores read constant tiles 4-7: no dep on the loads
                    for k in range(ng):
                        nc.sync.dma_start(
                            tiles[k % 4][:], xt[k * 128 : (k + 1) * 128, :]
                        )
                    for k in range(ng):
                        nc.sync.dma_start(
                            yt[k * 128 : (k + 1) * 128, :], tiles[4 + k % 4][:]
                        )
                elif mode == "ldst_i":
                    # fully interleaved single L/S pairs, independent tiles
                    for k in range(ng):
                        nc.sync.dma_start(
                            tiles[k % 4][:], xt[k * 128 : (k + 1) * 128, :]
                        )
                        nc.sync.dma_start(
                            yt[k * 128 : (k + 1) * 128, :], tiles[4 + k % 4][:]
                        )
                else:
                    for k in range(ng):
                        tle = tiles[k % nt]
                        if mode == "ld_only":
                            nc.sync.dma_start(tle[:], xt[k * 128 : (k + 1) * 128, :])
                        else:
                            nc.sync.dma_start(yt[k * 128 : (k + 1) * 128, :], tle[:])
        nc.compile()
        return nc

    with tile.TileContext(nc) as tc, ExitStack() as ctx:
        const_pool = ctx.enter_context(tc.tile_pool(name="const", bufs=1))
        small_pool = ctx.enter_context(tc.tile_pool(name="small", bufs=1))
        psum_prep = ctx.enter_context(tc.tile_pool(name="pprep", bufs=2, space="PSUM"))
        psum_bld = ctx.enter_context(tc.tile_pool(name="pbld", bufs=2, space="PSUM"))
        psum_main = ctx.enter_context(tc.tile_pool(name="pmain", bufs=4, space="PSUM"))
        xpool = ctx.enter_context(tc.tile_pool(name="xin", bufs=bufs))
        if obufs:
            opool = ctx.enter_context(tc.tile_pool(name="yout", bufs=obufs))

        # ---- small constant loads (all reads; precede the x-load burst)
        ident_sb = const_pool.tile([128, 128], f32)
        nc.sync.dma_start(ident_sb[:], ident[:])
        e1 = const_pool.tile([128, EMB], f32)
        nc.sync.dma_start(e1[:], emb[0:128, :])
        e2 = const_pool.tile([80, EMB], f32, tag="e2")
        nc.sync.dma_start(e2[:], emb[128:NODE_P, :])
        bias_sb = const_pool.tile([128, F], f32)
        nc.sync.dma_start(bias_sb[:], bias_d[:])
        eye1 = const_pool.tile([128, NODE], f32)
        nc.sync.dma_start(eye1[:], eye[0:128, :])
        eye2 = const_pool.tile([80, NODE], f32, tag="eye2")
        nc.sync.dma_start(eye2[:], eye[128:NODE_P, :])
        w_sb = const_pool.tile([128, F * C], bf16)
        nc.sync.dma_start(w_sb[:], w[:])

        # ---- E^T (unscaled) via PE transpose
        ET = small_pool.tile([128, NODE_P], f32)
        tp1 = psum_prep.tile([128, 128], f32, tag="prep")
        nc.tensor.transpose(tp1[:], e1[:], ident_sb[:])
        nc.vector.tensor_copy(ET[:, 0:128], tp1[:])
        tp2 = psum_prep.tile([128, 80], f32, tag="prep")
        nc.tensor.transpose(tp2[:], e2[:], ident_sb[0:80, 0:80])
        nc.vector.tensor_copy(ET[:, 128:NODE_P], tp2[:])

        # ---- sim = relu(E E^T) per row-tile; d = 1 + softmax diag
        def diag_gate(rows, off, e_tile, eye_tile):
            simp_t = psum_prep.tile([128, NODE_P], f32, tag="prep")
            simp = simp_t[0:rows, :]
            nc.tensor.matmul(simp, ET[:, off : off + rows], ET[:])
            s_t = small_pool.tile([128, NODE_P], f32, tag=f"s{off}")
            s = s_t[0:rows, :]
            nc.vector.tensor_relu(s[:], simp[:])
            m_t = small_pool.tile([128, 1], f32, tag=f"m{off}")
            m = m_t[0:rows, :]
            nc.vector.tensor_reduce(m[:], s[:, 0:NODE], AX.X, ALU.max)
            negm_t = small_pool.tile([128, 1], f32, tag=f"negm{off}")
            negm = negm_t[0:rows, :]
            nc.vector.tensor_scalar_mul(negm[:], m[:], -1.0)
            ex_t = small_pool.tile([128, NODE], f32, tag=f"ex{off}")
            ex = ex_t[0:rows, :]
            nc.scalar.activation(ex[:], s[:, 0:NODE], AF.Exp, bias=negm[:])
            z_t = small_pool.tile([128, 1], f32, tag=f"z{off}")
            z = z_t[0:rows, :]
            nc.vector.tensor_reduce(z[:], ex[:], AX.X, ALU.add)
            msk_t = small_pool.tile([128, NODE], f32, tag=f"msk{off}")
            msk = msk_t[0:rows, :]
            nc.vector.tensor_mul(msk[:], s[:, 0:NODE], eye_tile[:])
            dg_t = small_pool.tile([128, 1], f32, tag=f"dg{off}")
            dg = dg_t[0:rows, :]
            nc.vector.tensor_reduce(dg[:], msk[:], AX.X, ALU.add)
            ed_t = small_pool.tile([128, 1], f32, tag=f"ed{off}")
            ed = ed_t[0:rows, :]
            nc.scalar.activation(ed[:], dg[:], AF.Exp, bias=negm[:])
            rz_t = small_pool.tile([128, 1], f32, tag=f"rz{off}")
            rz = rz_t[0:rows, :]
            nc.vector.reciprocal(rz[:], z[:])
            d_t = small_pool.tile([128, 1], f32, tag=f"d{off}")
            d = d_t[0:rows, :]
            nc.vector.tensor_mul(d[:], ed[:], rz[:])
            nc.vector.tensor_scalar_add(d[:], d[:], 1.0)
            # E' = diag(d) @ E rows
            ep_t = small_pool.tile([128, EMB], f32, tag=f"ep{off}")
            ep = ep_t[0:rows, :]
            nc.vector.tensor_scalar_mul(ep[:], e_tile[:], d[:])
            return ep

        ep1 = diag_gate(128, 0, e1, eye1)
        ep2 = diag_gate(80, 128, e2, eye2)

        # ---- E'^T via PE transpose, narrowed to bf16 for the Ew build
        EpTb = small_pool.tile([128, NODE_P], bf16)
        tq1 = psum_prep.tile([128, 128], f32, tag="prep")
        nc.tensor.transpose(tq1[:], ep1[:], ident_sb[:])
        nc.vector.tensor_copy(EpTb[:, 0:128], tq1[:])
        tq2 = psum_prep.tile([128, 80], f32, tag="prep")
        nc.tensor.transpose(tq2[:], ep2[:], ident_sb[0:80, 0:80])
        nc.vector.tensor_copy(EpTb[:, 128:NODE_P], tq2[:])

        # ---- per-node bias, pair-stacked: ebT2[64*par + f, p] = eb[2p+par, f]
        ebT2 = const_pool.tile([128, PAIRS], f32)
        pe = psum_prep.tile([128, PAIRS], f32, tag="prep")
        nc.tensor.matmul(pe[0:64, :], bias_sb[:], ET[:, 0:NODE_P:2])
        nc.tensor.matmul(pe[64:128, :], bias_sb[:], ET[:, 1:NODE_P:2])
        nc.vector.tensor_copy(ebT2[:], pe[:])

        # ---- stationary weights, block-diagonal per pair (bf16):
        #   Ew[c,       p*128 + f]      = sum_d E'[2p,   d] W[d, c, f]
        #   Ew[64 + c,  p*128 + 64 + f] = sum_d E'[2p+1, d] W[d, c, f]
        # (off-diagonal quadrants stay zero)
        Ew = const_pool.tile([128, 128 * PAIRS], bf16)
        zsrc = small_pool.tile([128, 8 * PAIRS], f32)
        nc.vector.memset(zsrc[:], 0.0)
        Ew3 = Ew[:].rearrange("p (q b) -> p q b", b=128)
        zs3 = zsrc[:].rearrange("p (q b) -> p q b", b=8)
        for s in range(8):
            nc.vector.tensor_copy(Ew3[0:64, :, 64 + 8 * s : 72 + 8 * s], zs3[0:64, :, :])
            nc.vector.tensor_copy(Ew3[64:128, :, 8 * s : 8 * (s + 1)], zs3[64:128, :, :])
        for f in range(F):
            # dedicated double-buffered pool so matmul f+1 overlaps the
            # copies of f instead of serializing the whole build
            pf = psum_bld.tile([128, PAIRS], f32)
            wf = w_sb[:, f * C : (f + 1) * C]  # W[:, f, :] (w is f-major from host)
            nc.tensor.matmul(pf[0:64, :], wf, EpTb[:, 0:NODE_P:2])
            nc.tensor.matmul(pf[64:128, :], wf, EpTb[:, 1:NODE_P:2])
            if f % 2 == 0:
                nc.vector.tensor_copy(Ew[0:64, f :: 128], pf[0:64, :])
                nc.vector.tensor_copy(Ew[64:128, (64 + f) :: 128], pf[64:128, :])
            else:
                nc.scalar.copy(Ew[0:64, f :: 128], pf[0:64, :])
                nc.scalar.copy(Ew[64:128, (64 + f) :: 128], pf[64:128, :])

        # ---- main streaming loop over node pairs, burst-phased on ONE HWDGE
        # ring: the ring's FIFO keeps the load burst and store burst of each
        # phase apart, so HBM traffic stays unidirectional per burst (mixed
        # read/write collapses DMA throughput ~2x, measured)
        st_eng = nc.sync if ring == "sync" else nc.scalar

        def main_loop():
            for k0 in range(0, ng, ph):
                phase = []
                for k in range(k0, min(k0 + ph, ng)):
                    x2 = xpool.tile([128, nb * R], bf16)
                    nc.sync.dma_start(x2[:], xt[k * 128 : (k + 1) * 128, :])
                    phase.append((k, x2))
                for k, x2 in phase:
                    # obufs>0: bias-adds write a separate staging tile so the
                    # x2 tile is freed by the PE's last read (engine sem) and
                    # the next pass's load never waits on a store completion
                    out = opool.tile([128, nb * R], bf16, name="out") if obufs else x2
                    for j in range(nb):
                        p = k * nb + j
                        ew_p = Ew[:, p * 128 : (p + 1) * 128]
                        for h in range(2):
                            ps = psum_main.tile([128, RH], f32)
                            cols = slice(j * R + h * RH, j * R + (h + 1) * RH)
                            nc.tensor.matmul(ps[:], ew_p, x2[:, cols])
                            # alternate DVE / ACT so neither engine bottlenecks
                            if h == 0:
                                nc.vector.tensor_scalar_add(
                                    out[:, cols], ps[:], ebT2[:, p : p + 1]
                                )
                            else:
                                nc.scalar.add(out[:, cols], ps[:], ebT2[:, p : p + 1])
                    st_eng.dma_start(yt[k * 128 : (k + 1) * 128, :], out[:])

        if repeat == 1:
            main_loop()
        else:
            # hardware loop: one NEFF execution = `repeat` full streaming passes
            # (benchmarking only; kernel() uses repeat=1)
            with tc.For_i(0, repeat, 1):
                main_loop()

    nc.compile()
    return nc


def _get_nc(
    repeat=1,
    bufs=None,
    ph=None,
    nb=NB,
    ring="sync",
    mode="normal",
    ncores=N_CORES,
    obufs=0,
):
    key = f"nc{repeat}_{bufs}_{ph}_{nb}_{ring}_{mode}_{ncores}_{obufs}"
    if key not in _CACHE:
        _CACHE[key] = _build(repeat, bufs, ph, nb, ring, mode, ncores, obufs)
    return _CACHE[key]


def host_in_maps(x, node_embedding, weights, bias, nb=NB):
    """Shard + permute full inputs into per-core input maps (bf16 wire)."""
    import ml_dtypes

    bf = ml_dtypes.bfloat16
    ng = PAIRS // nb

    emb_p = np.zeros((NODE_P, EMB), np.float32)
    emb_p[:NODE] = node_embedding
    w2 = np.ascontiguousarray(
        np.asarray(weights).transpose(0, 2, 1).reshape(EMB, F * C)
    ).astype(bf)
    bias_f = np.ascontiguousarray(bias, np.float32)
    eye_np = np.eye(NODE_P, NODE, dtype=np.float32)
    ident_np = np.eye(128, dtype=np.float32)

    in_maps = []
    for i in range(N_CORES):
        xi = x[B_SH * i : B_SH * (i + 1)]  # [2, T, NODE, C]
        xt = np.zeros((NODE_P, C, R), bf)
        xt[:NODE] = np.asarray(xi).transpose(2, 3, 0, 1).reshape(NODE, C, R).astype(bf)
        # group nb pairs: [G, nb, 2, C, R] -> [G, (2, C)=128, nb, R] so each
        # partition line is one contiguous nb*R*2-byte DMA descriptor
        xt_g = np.ascontiguousarray(
            xt.reshape(ng, nb, 2, C, R).transpose(0, 2, 3, 1, 4)
        ).reshape(ng * 128, nb * R)
        in_maps.append(
            {
                "xt": xt_g,
                "emb": emb_p,
                "w": w2,
                "bias": bias_f,
                "eye": eye_np,
                "ident": ident_np,
            }
        )
    return in_maps


def host_out(results, nb=NB):
    """Un-permute per-core yt shards back to the full [B, T, N, F] f32 output."""
    ng = PAIRS // nb
    out = np.empty((B, T, NODE, F), np.float32)
    for i in range(N_CORES):
        ytr = results[i]["yt"].reshape(ng, 2, F, nb, B_SH, T).astype(np.float32)
        y_local = ytr.transpose(4, 5, 0, 3, 1, 2).reshape(B_SH, T, NODE_P, F)
        out[B_SH * i : B_SH * (i + 1)] = y_local[:, :, :NODE, :]
    return out


def _spot_reference(x, node_embedding, weights, bias, n_rows=2):
    """Reference y[0, :n_rows] in f32 numpy — cheap guard against transient
    garbage from a wedged device (observed once: whole-output corruption)."""
    E = np.asarray(node_embedding, np.float64)
    sim = np.maximum(E @ E.T, 0.0)
    ex = np.exp(sim - sim.max(axis=1, keepdims=True))
    d = 1.0 + np.diag(ex / ex.sum(axis=1, keepdims=True))
    Ew = np.einsum(
        "nd,dcf->ncf", d[:, None] * E, np.asarray(weights, np.float64)
    )
    eb = E @ np.asarray(bias, np.float64)
    xs = np.asarray(x[0, :n_rows], np.float64)  # [n_rows, NODE, C]
    return (np.einsum("tnc,ncf->tnf", xs, Ew) + eb).astype(np.float32)


def kernel(x, node_embedding, weights, bias):
    from concourse.bass_utils import run_bass_kernel_spmd

    nc = _get_nc()
    in_maps = host_in_maps(x, node_embedding, weights, bias)
    y_spot = _spot_reference(x, node_embedding, weights, bias)
    scale = np.abs(y_spot).max() + 1e-30
    for attempt in range(2):
        res = run_bass_kernel_spmd(nc, in_maps, core_ids=list(range(N_CORES)))
        out = host_out(res.results)
        err = np.abs(out[0, : y_spot.shape[0]] - y_spot).max() / scale
        if err < 0.05:  # bf16 wire error is ~5e-3; garbage is >>1
            break
    return out


# revision 41
# speedup vs baseline: 1.0588x; 1.0588x over previous
"""AdaptiveGraphConv Trainium2 kernel, data-parallel over batch on 8 NeuronCores.

Reference computation (per full input):
  sim  = relu(E @ E^T)                               [N, N]
  d[n] = 1 + softmax(sim, axis=1)[n, n]              (diag gate)
  Ew   = einsum('nd,dcf->ncf', E, W)                 per-node weights
  eb   = E @ bias                                    per-node bias [N, F]
  y[b,t,n,f] = (d[n] * x[b,t,n,:]) @ Ew[n] + eb[n]

Device strategy per core (2 of 16 batches, R = 2*288 = 576 rows):
  - host supplies x transposed to node-major [N, C, R] in BF16 so each
    node-pair is a contiguous [128, 576] block (C on partitions = matmul
    contraction layout). BF16 on the wire halves the HBM traffic that bounds
    this kernel (~358 GB/s/core HBM limit); rel-err budget 2e-2 vs ~4.4e-3
    measured for bf16 x/Ew/y with f32 PSUM accumulation.
  - host packs NB=13 pairs per DMA group so each SBUF partition line is one
    contiguous 15 KB descriptor: with bf16's 1.15 KB/pair lines the DMA is
    descriptor-rate-bound (~5.7 ns/desc ~ 200 GB/s measured); grouping
    restores byte-bound streaming.
  - on-chip: compute d, fold it into E (E' = diag(d) @ E), build
    block-diagonal per-pair stationary weights Ew in BF16, then stream 8
    groups x 13 node-pairs: 2 bf16 matmuls per pair (row halves), bias-add
    from PSUM written in place over the x tile, alternating DVE / ACT per
    half so neither engine bottlenecks. Loads and stores ride ONE HWDGE ring
    in whole-pass bursts so HBM traffic stays unidirectional per burst
    (measured here: pure reads 700 GB/s/core, pure writes 676 GB/s/core,
    but any read/write alternation collapses the combined rate to ~300-450
    GB/s; separate concurrent load/store rings are worse still).
  - host un-permutes y^T shards back to [B, T, N, F] and widens to f32.
"""

import sys

sys.path.insert(0, "/opt/trn_rl_repo")

from contextlib import ExitStack

import numpy as np

N_CORES = 8
NODE = 207
NODE_P = 208  # padded to even node count
PAIRS = NODE_P // 2  # 104
EMB = 128
C = 64
F = 64
B = 16
T = 288
B_SH = B // N_CORES  # 2
R = B_SH * T  # 576 rows per core
RH = R // 2  # 288, matmul free-dim chunk
NB = 13  # pairs per DMA group (contiguous per-partition lines)
G = PAIRS // NB  # 8 groups

_CACHE = {}


def _build(
    repeat=1,
    bufs=None,
    ph=None,
    nb=NB,
    ring="sync",
    mode="normal",
    ncores=N_CORES,
    obufs=0,
):
    ng = PAIRS // nb
    if bufs is None:
        # +2 slack buffers: the next pass's load then WARs on a store two
        # positions back (already complete), instead of stalling the ring
        bufs = ng if mode != "normal" else ng + 2
    if ph is None:
        ph = ng
    import concourse.tile as tile
    from concourse import bacc, mybir

    f32 = mybir.dt.float32
    bf16 = mybir.dt.bfloat16
    AF = mybir.ActivationFunctionType
    ALU = mybir.AluOpType
    AX = mybir.AxisListType

    nc = bacc.Bacc("TRN2", target_bir_lowering=False, debug=False, num_devices=ncores)
    xt = nc.dram_tensor("xt", [ng * 128, nb * R], bf16, kind="ExternalInput").ap()
    emb = nc.dram_tensor("emb", [NODE_P, EMB], f32, kind="ExternalInput").ap()
    w = nc.dram_tensor("w", [EMB, F * C], bf16, kind="ExternalInput").ap()
    bias_d = nc.dram_tensor("bias", [EMB, F], f32, kind="ExternalInput").ap()
    eye = nc.dram_tensor("eye", [NODE_P, NODE], f32, kind="ExternalInput").ap()
    ident = nc.dram_tensor("ident", [128, 128], f32, kind="ExternalInput").ap()
    yt = nc.dram_tensor("yt", [ng * 128, nb * R], bf16, kind="ExternalOutput").ap()

    if mode in ("ld_only", "st_only", "ldst", "ldst2", "ldst_i"):
        # bandwidth diagnostics: raw DMA streams, no compute
        nt = 8 if mode in ("ldst2", "ldst_i") else bufs
        with tile.TileContext(nc) as tc, ExitStack() as ctx:
            dpool = ctx.enter_context(tc.tile_pool(name="dbw", bufs=1))
            tiles = [
                dpool.tile([128, nb * R], bf16, name=f"bw{i}", tag=f"bw{i}")
                for i in range(nt)
            ]
            for tle in tiles:
                nc.vector.memset(tle[:].bitcast(f32), 0.0)
            if mode == "ld_only":
                # the declared output must be written at least once
                nc.sync.dma_start(yt[0:128, :], tiles[0][:])
            # software-unrolled repeat (For_i needs every engine in the body)
            for _ in range(repeat):
                if mode == "ldst":
                    for k in range(ng):
                        nc.sync.dma_start(
                            tiles[k % nt][:], xt[k * 128 : (k + 1) * 128, :]
                        )
                    for k in range(ng):
                        nc.sync.dma_start(
                            yt[k * 128 : (k + 1) * 128, :], tiles[k % nt][:]
                        )
                elif mode == "ldst2":
                    # stores read constant tiles 4-7: no dep on the loads
                    for k in range(ng):
                        nc.sync.dma_start(
                            tiles[k % 4][:], xt[k * 128 : (k + 1) * 128, :]
                        )
                    for k in range(ng):
                        nc.sync.dma_start(
                            yt[k * 128 : (k + 1) * 128, :], tiles[4 + k % 4][:]
                        )
                elif mode == "ldst_i":
                    # fully interleaved single L/S pairs, independent tiles
                    for k in range(ng):
                        nc.sync.dma_start(
                            tiles[k % 4][:], xt[k * 128 : (k + 1) * 128, :]
                        )
                        nc.sync.dma_start(
                            yt[k * 128 : (k + 1) * 128, :], tiles[4 + k % 4][:]
                        )
                else:
                    for k in range(ng):
                        tle = tiles[k % nt]
                        if mode == "ld_only":
                            nc.sync.dma_start(tle[:], xt[k * 128 : (k + 1) * 128, :])
                        else:
                            nc.sync.dma_start(yt[k * 128 : (k + 1) * 128, :], tle[:])
        nc.compile()
        return nc

    with tile.TileContext(nc) as tc, ExitStack() as ctx:
        const_pool = ctx.enter_context(tc.tile_pool(name="const", bufs=1))
        small_pool = ctx.enter_context(tc.tile_pool(name="small", bufs=1))
        psum_prep = ctx.enter_context(tc.tile_pool(name="pprep", bufs=2, space="PSUM"))
        psum_bld = ctx.enter_context(tc.tile_pool(name="pbld", bufs=2, space="PSUM"))
        psum_main = ctx.enter_context(tc.tile_pool(name="pmain", bufs=4, space="PSUM"))
        xpool = ctx.enter_context(tc.tile_pool(name="xin", bufs=bufs))
        if obufs:
            opool = ctx.enter_context(tc.tile_pool(name="yout", bufs=obufs))

        # ---- small constant loads (all reads; precede the x-load burst)
        ident_sb = const_pool.tile([128, 128], f32)
        nc.sync.dma_start(ident_sb[:], ident[:])
        e1 = const_pool.tile([128, EMB], f32)
        nc.sync.dma_start(e1[:], emb[0:128, :])
        e2 = const_pool.tile([80, EMB], f32, tag="e2")
        nc.sync.dma_start(e2[:], emb[128:NODE_P, :])
        bias_sb = const_pool.tile([128, F], f32)
        nc.sync.dma_start(bias_sb[:], bias_d[:])
        eye1 = const_pool.tile([128, NODE], f32)
        nc.sync.dma_start(eye1[:], eye[0:128, :])
        eye2 = const_pool.tile([80, NODE], f32, tag="eye2")
        nc.sync.dma_start(eye2[:], eye[128:NODE_P, :])
        w_sb = const_pool.tile([128, F * C], bf16)
        nc.sync.dma_start(w_sb[:], w[:])

        # ---- E^T (unscaled) via PE transpose
        ET = small_pool.tile([128, NODE_P], f32)
        tp1 = psum_prep.tile([128, 128], f32, tag="prep")
        nc.tensor.transpose(tp1[:], e1[:], ident_sb[:])
        nc.vector.tensor_copy(ET[:, 0:128], tp1[:])
        tp2 = psum_prep.tile([128, 80], f32, tag="prep")
        nc.tensor.transpose(tp2[:], e2[:], ident_sb[0:80, 0:80])
        nc.vector.tensor_copy(ET[:, 128:NODE_P], tp2[:])

        # ---- sim = relu(E E^T) per row-tile; d = 1 + softmax diag
        def diag_gate(rows, off, e_tile, eye_tile):
            simp_t = psum_prep.tile([128, NODE_P], f32, tag="prep")
            simp = simp_t[0:rows, :]
            nc.tensor.matmul(simp, ET[:, off : off + rows], ET[:])
            s_t = small_pool.tile([128, NODE_P], f32, tag=f"s{off}")
            s = s_t[0:rows, :]
            nc.vector.tensor_relu(s[:], simp[:])
            m_t = small_pool.tile([128, 1], f32, tag=f"m{off}")
            m = m_t[0:rows, :]
            nc.vector.tensor_reduce(m[:], s[:, 0:NODE], AX.X, ALU.max)
            negm_t = small_pool.tile([128, 1], f32, tag=f"negm{off}")
            negm = negm_t[0:rows, :]
            nc.vector.tensor_scalar_mul(negm[:], m[:], -1.0)
            ex_t = small_pool.tile([128, NODE], f32, tag=f"ex{off}")
            ex = ex_t[0:rows, :]
            nc.scalar.activation(ex[:], s[:, 0:NODE], AF.Exp, bias=negm[:])
            z_t = small_pool.tile([128, 1], f32, tag=f"z{off}")
            z = z_t[0:rows, :]
            nc.vector.tensor_reduce(z[:], ex[:], AX.X, ALU.add)
            msk_t = small_pool.tile([128, NODE], f32, tag=f"msk{off}")
            msk = msk_t[0:rows, :]
            nc.vector.tensor_mul(msk[:], s[:, 0:NODE], eye_tile[:])
            dg_t = small_pool.tile([128, 1], f32, tag=f"dg{off}")
            dg = dg_t[0:rows, :]
            nc.vector.tensor_reduce(dg[:], msk[:], AX.X, ALU.add)
            ed_t = small_pool.tile([128, 1], f32, tag=f"ed{off}")
            ed = ed_t[0:rows, :]
            nc.scalar.activation(ed[:], dg[:], AF.Exp, bias=negm[:])
            rz_t = small_pool.tile([128, 1], f32, tag=f"rz{off}")
            rz = rz_t[0:rows, :]
            nc.vector.reciprocal(rz[:], z[:])
            d_t = small_pool.tile([128, 1], f32, tag=f"d{off}")
            d = d_t[0:rows, :]
            nc.vector.tensor_mul(d[:], ed[:], rz[:])
            nc.vector.tensor_scalar_add(d[:], d[:], 1.0)
            # E' = diag(d) @ E rows
            ep_t = small_pool.tile([128, EMB], f32, tag=f"ep{off}")
            ep = ep_t[0:rows, :]
            nc.vector.tensor_scalar_mul(ep[:], e_tile[:], d[:])
            return ep

        ep1 = diag_gate(128, 0, e1, eye1)
        ep2 = diag_gate(80, 128, e2, eye2)

        # ---- E'^T via PE transpose, narrowed to bf16 for the Ew build
        EpTb = small_pool.tile([128, NODE_P], bf16)
        tq1 = psum_prep.tile([128, 128], f32, tag="prep")
        nc.tensor.transpose(tq1[:], ep1[:], ident_sb[:])
        nc.vector.tensor_copy(EpTb[:, 0:128], tq1[:])
        tq2 = psum_prep.tile([128, 80], f32, tag="prep")
        nc.tensor.transpose(tq2[:], ep2[:], ident_sb[0:80, 0:80])
        nc.vector.tensor_copy(EpTb[:, 128:NODE_P], tq2[:])

        # ---- per-node bias, pair-stacked: ebT2[64*par + f, p] = eb[2p+par, f]
        ebT2 = const_pool.tile([128, PAIRS], f32)
        pe = psum_prep.tile([128, PAIRS], f32, tag="prep")
        nc.tensor.matmul(pe[0:64, :], bias_sb[:], ET[:, 0:NODE_P:2])
        nc.tensor.matmul(pe[64:128, :], bias_sb[:], ET[:, 1:NODE_P:2])
        nc.vector.tensor_copy(ebT2[:], pe[:])

        # ---- stationary weights, block-diagonal per pair (bf16):
        #   Ew[c,       p*128 + f]      = sum_d E'[2p,   d] W[d, c, f]
        #   Ew[64 + c,  p*128 + 64 + f] = sum_d E'[2p+1, d] W[d, c, f]
        # (off-diagonal quadrants stay zero)
        Ew = const_pool.tile([128, 128 * PAIRS], bf16)
        zsrc = small_pool.tile([128, 8 * PAIRS], f32)
        nc.vector.memset(zsrc[:], 0.0)
        Ew3 = Ew[:].rearrange("p (q b) -> p q b", b=128)
        zs3 = zsrc[:].rearrange("p (q b) -> p q b", b=8)
        for s in range(8):
            nc.vector.tensor_copy(Ew3[0:64, :, 64 + 8 * s : 72 + 8 * s], zs3[0:64, :, :])
            nc.vector.tensor_copy(Ew3[64:128, :, 8 * s : 8 * (s + 1)], zs3[64:128, :, :])
        for f in range(F):
            # dedicated double-buffered pool so matmul f+1 overlaps the
            # copies of f instead of serializing the whole build
            pf = psum_bld.tile([128, PAIRS], f32)
            wf = w_sb[:, f * C : (f + 1) * C]  # W[:, f, :] (w is f-major from host)
            nc.tensor.matmul(pf[0:64, :], wf, EpTb[:, 0:NODE_P:2])
            nc.tensor.matmul(pf[64:128, :], wf, EpTb[:, 1:NODE_P:2])
            if f % 2 == 0:
                nc.vector.tensor_copy(Ew[0:64, f :: 128], pf[0:64, :])
                nc.vector.tensor_copy(Ew[64:128, (64 + f) :: 128], pf[64:128, :])
            else:
                nc.scalar.copy(Ew[0:64, f :: 128], pf[0:64, :])
                nc.scalar.copy(Ew[64:128, (64 + f) :: 128], pf[64:128, :])

        # ---- main streaming loop over node pairs, burst-phased on ONE HWDGE
        # ring: the ring's FIFO keeps the load burst and store burst of each
        # phase apart, so HBM traffic stays unidirectional per burst (mixed
        # read/write collapses DMA throughput ~2x, measured)
        st_eng = nc.sync if ring == "sync" else nc.scalar

        def main_loop():
            for k0 in range(0, ng, ph):
                phase = []
                for k in range(k0, min(k0 + ph, ng)):
                    x2 = xpool.tile([128, nb * R], bf16)
                    nc.sync.dma_start(x2[:], xt[k * 128 : (k + 1) * 128, :])
                    phase.append((k, x2))
                for k, x2 in phase:
                    # obufs>0: bias-adds write a separate staging tile so the
                    # x2 tile is freed by the PE's last read (engine sem) and
                    # the next pass's load never waits on a store completion
                    out = opool.tile([128, nb * R], bf16, name="out") if obufs else x2
                    for j in range(nb):
                        p = k * nb + j
                        ew_p = Ew[:, p * 128 : (p + 1) * 128]
                        for h in range(2):
                            ps = psum_main.tile([128, RH], f32)
                            cols = slice(j * R + h * RH, j * R + (h + 1) * RH)
                            nc.tensor.matmul(ps[:], ew_p, x2[:, cols])
                            # alternate DVE / ACT so neither engine bottlenecks
                            if h == 0:
                                nc.vector.tensor_scalar_add(
                                    out[:, cols], ps[:], ebT2[:, p : p + 1]
                                )
                            else:
                                nc.scalar.add(out[:, cols], ps[:], ebT2[:, p : p + 1])
                    st_eng.dma_start(yt[k * 128 : (k + 1) * 128, :], out[:])

        if repeat == 1:
            main_loop()
        else:
            # hardware loop: one NEFF execution = `repeat` full streaming passes
            # (benchmarking only; kernel() uses repeat=1)
            with tc.For_i(0, repeat, 1):
                main_loop()

    nc.compile()
    return nc


def _get_nc(
    repeat=1,
    bufs=None,
    ph=None,
    nb=NB,
    ring="sync",
    mode="normal",
    ncores=N_CORES,
    obufs=0,
):
    key = f"nc{repeat}_{bufs}_{ph}_{nb}_{ring}_{mode}_{ncores}_{obufs}"
    if key not in _CACHE:
        _CACHE[key] = _build(repeat, bufs, ph, nb, ring, mode, ncores, obufs)
    return _CACHE[key]


def host_in_maps(x, node_embedding, weights, bias, nb=NB):
    """Shard + permute full inputs into per-core input maps (bf16 wire)."""
    import ml_dtypes

    bf = ml_dtypes.bfloat16
    ng = PAIRS // nb

    emb_p = np.zeros((NODE_P, EMB), np.float32)
    emb_p[:NODE] = node_embedding
    w2 = np.ascontiguousarray(
        np.asarray(weights).transpose(0, 2, 1).reshape(EMB, F * C)
    ).astype(bf)
    bias_f = np.ascontiguousarray(bias, np.float32)
    eye_np = np.eye(NODE_P, NODE, dtype=np.float32)
    ident_np = np.eye(128, dtype=np.float32)

    in_maps = []
    for i in range(N_CORES):
        xi = x[B_SH * i : B_SH * (i + 1)]  # [2, T, NODE, C]
        xt = np.zeros((NODE_P, C, R), bf)
        xt[:NODE] = np.asarray(xi).transpose(2, 3, 0, 1).reshape(NODE, C, R).astype(bf)
        # group nb pairs: [G, nb, 2, C, R] -> [G, (2, C)=128, nb, R] so each
        # partition line is one contiguous nb*R*2-byte DMA descriptor
        xt_g = np.ascontiguousarray(
            xt.reshape(ng, nb, 2, C, R).transpose(0, 2, 3, 1, 4)
        ).reshape(ng * 128, nb * R)
        in_maps.append(
            {
                "xt": xt_g,
                "emb": emb_p,
                "w": w2,
                "bias": bias_f,
                "eye": eye_np,
                "ident": ident_np,
            }
        )
    return in_maps


def host_out(results, nb=NB):
    """Un-permute per-core yt shards back to the full [B, T, N, F] f32 output."""
    ng = PAIRS // nb
    out = np.empty((B, T, NODE, F), np.float32)
    for i in range(N_CORES):
        ytr = results[i]["yt"].reshape(ng, 2, F, nb, B_SH, T).astype(np.float32)
        y_local = ytr.transpose(4, 5, 0, 3, 1, 2).reshape(B_SH, T, NODE_P, F)
        out[B_SH * i : B_SH * (i + 1)] = y_local[:, :, :NODE, :]
    return out


def _spot_reference(x, node_embedding, weights, bias, n_rows=2):
    """Reference y[0, :n_rows] in f32 numpy — cheap guard against transient
    garbage from a wedged device (observed once: whole-output corruption)."""
    E = np.asarray(node_embedding, np.float64)
    sim = np.maximum(E @ E.T, 0.0)
    ex = np.exp(sim - sim.max(axis=1, keepdims=True))
    d = 1.0 + np.diag(ex / ex.sum(axis=1, keepdims=True))
    Ew = np.einsum(
        "nd,dcf->ncf", d[:, None] * E, np.asarray(weights, np.float64)
    )
    eb = E @ np.asarray(bias, np.float64)
    xs = np.asarray(x[0, :n_rows], np.float64)  # [n_rows, NODE, C]
    return (np.einsum("tnc,ncf->tnf", xs, Ew) + eb).astype(np.float32)


def kernel(x, node_embedding, weights, bias):
    from concourse.bass_utils import run_bass_kernel_spmd

    nc = _get_nc()
    in_maps = host_in_maps(x, node_embedding, weights, bias)
    y_spot = _spot_reference(x, node_embedding, weights, bias)
    scale = np.abs(y_spot).max() + 1e-30
    for attempt in range(2):
        res = run_bass_kernel_spmd(nc, in_maps, core_ids=list(range(N_CORES)))
        out = host_out(res.results)
        err = np.abs(out[0, : y_spot.shape[0]] - y_spot).max() / scale
        if err < 0.05:  # bf16 wire error is ~5e-3; garbage is >>1
            break
    return out


# revision 44
# speedup vs baseline: 1.0912x; 1.0306x over previous
"""AdaptiveGraphConv Trainium2 kernel, data-parallel over batch on 8 NeuronCores.

Reference computation (per full input):
  sim  = relu(E @ E^T)                               [N, N]
  d[n] = 1 + softmax(sim, axis=1)[n, n]              (diag gate)
  Ew   = einsum('nd,dcf->ncf', E, W)                 per-node weights
  eb   = E @ bias                                    per-node bias [N, F]
  y[b,t,n,f] = (d[n] * x[b,t,n,:]) @ Ew[n] + eb[n]

Device strategy per core (2 of 16 batches, R = 2*288 = 576 rows):
  - host supplies x transposed to node-major [N, C, R] in BF16 so each
    node-pair is a contiguous [128, 576] block (C on partitions = matmul
    contraction layout). BF16 on the wire halves the HBM traffic that bounds
    this kernel (~358 GB/s/core HBM limit); rel-err budget 2e-2 vs ~4.4e-3
    measured for bf16 x/Ew/y with f32 PSUM accumulation.
  - host packs NB=13 pairs per DMA group so each SBUF partition line is one
    contiguous 15 KB descriptor: with bf16's 1.15 KB/pair lines the DMA is
    descriptor-rate-bound (~5.7 ns/desc ~ 200 GB/s measured); grouping
    restores byte-bound streaming.
  - on-chip: compute d, fold it into E (E' = diag(d) @ E), build
    block-diagonal per-pair stationary weights Ew in BF16, then stream 8
    groups x 13 node-pairs: 2 bf16 matmuls per pair (row halves), bias-add
    from PSUM written in place over the x tile, alternating DVE / ACT per
    half so neither engine bottlenecks. Loads and stores ride ONE HWDGE ring
    in whole-pass bursts so HBM traffic stays unidirectional per burst
    (measured here: pure reads 700 GB/s/core, pure writes 676 GB/s/core,
    but any read/write alternation collapses the combined rate to ~300-450
    GB/s; separate concurrent load/store rings are worse still).
  - host un-permutes y^T shards back to [B, T, N, F] and widens to f32.
"""

import sys

sys.path.insert(0, "/opt/trn_rl_repo")

from contextlib import ExitStack

import numpy as np

N_CORES = 8
NODE = 207
NODE_P = 208  # padded to even node count
PAIRS = NODE_P // 2  # 104
EMB = 128
C = 64
F = 64
B = 16
T = 288
B_SH = B // N_CORES  # 2
R = B_SH * T  # 576 rows per core
RH = R // 2  # 288, matmul free-dim chunk
NB = 13  # pairs per DMA group (contiguous per-partition lines)
G = PAIRS // NB  # 8 groups

_CACHE = {}


def _build(
    repeat=1,
    bufs=None,
    ph=None,
    nb=NB,
    ring="sync",
    mode="normal",
    ncores=N_CORES,
    obufs=0,
):
    ng = PAIRS // nb
    if bufs is None:
        # +2 slack buffers: the next pass's load then WARs on a store two
        # positions back (already complete), instead of stalling the ring
        bufs = ng if mode != "normal" else ng + 2
    if ph is None:
        ph = ng
    import concourse.tile as tile
    from concourse import bacc, mybir

    f32 = mybir.dt.float32
    bf16 = mybir.dt.bfloat16
    AF = mybir.ActivationFunctionType
    ALU = mybir.AluOpType
    AX = mybir.AxisListType

    nc = bacc.Bacc("TRN2", target_bir_lowering=False, debug=False, num_devices=ncores)
    xt = nc.dram_tensor("xt", [ng * 128, nb * R], bf16, kind="ExternalInput").ap()
    emb = nc.dram_tensor("emb", [NODE_P, EMB], f32, kind="ExternalInput").ap()
    w = nc.dram_tensor("w", [EMB, F * C], bf16, kind="ExternalInput").ap()
    bias_d = nc.dram_tensor("bias", [EMB, F], f32, kind="ExternalInput").ap()
    eye = nc.dram_tensor("eye", [NODE_P, NODE], f32, kind="ExternalInput").ap()
    ident = nc.dram_tensor("ident", [128, 128], f32, kind="ExternalInput").ap()
    yt = nc.dram_tensor("yt", [ng * 128, nb * R], bf16, kind="ExternalOutput").ap()

    if mode in ("ld_only", "st_only", "ldst", "ldst2", "ldst_i"):
        # bandwidth diagnostics: raw DMA streams, no compute
        nt = 8 if mode in ("ldst2", "ldst_i") else bufs
        with tile.TileContext(nc) as tc, ExitStack() as ctx:
            dpool = ctx.enter_context(tc.tile_pool(name="dbw", bufs=1))
            tiles = [
                dpool.tile([128, nb * R], bf16, name=f"bw{i}", tag=f"bw{i}")
                for i in range(nt)
            ]
            for tle in tiles:
                nc.vector.memset(tle[:].bitcast(f32), 0.0)
            if mode == "ld_only":
                # the declared output must be written at least once
                nc.sync.dma_start(yt[0:128, :], tiles[0][:])
            # software-unrolled repeat (For_i needs every engine in the body)
            for _ in range(repeat):
                if mode == "ldst":
                    for k in range(ng):
                        nc.sync.dma_start(
                            tiles[k % nt][:], xt[k * 128 : (k + 1) * 128, :]
                        )
                    for k in range(ng):
                        nc.sync.dma_start(
                            yt[k * 128 : (k + 1) * 128, :], tiles[k % nt][:]
                        )
                elif mode == "ldst2":
                    # stores read constant tiles 4-7: no dep on the loads
                    for k in range(ng):
                        nc.sync.dma_start(
                            tiles[k % 4][:], xt[k * 128 : (k + 1) * 128, :]
                        )
                    for k in range(ng):
                        nc.sync.dma_start(
                            yt[k * 128 : (k + 1) * 128, :], tiles[4 + k % 4][:]
                        )
                elif mode == "ldst_i":
                    # fully interleaved single L/S pairs, independent tiles
                    for k in range(ng):
                        nc.sync.dma_start(
                            tiles[k % 4][:], xt[k * 128 : (k + 1) * 128, :]
                        )
                        nc.sync.dma_start(
                            yt[k * 128 : (k + 1) * 128, :], tiles[4 + k % 4][:]
                        )
                else:
                    for k in range(ng):
                        tle = tiles[k % nt]
                        if mode == "ld_only":
                            nc.sync.dma_start(tle[:], xt[k * 128 : (k + 1) * 128, :])
                        else:
                            nc.sync.dma_start(yt[k * 128 : (k + 1) * 128, :], tle[:])
        nc.compile()
        return nc

    with tile.TileContext(nc) as tc, ExitStack() as ctx:
        const_pool = ctx.enter_context(tc.tile_pool(name="const", bufs=1))
        small_pool = ctx.enter_context(tc.tile_pool(name="small", bufs=1))
        psum_prep = ctx.enter_context(tc.tile_pool(name="pprep", bufs=2, space="PSUM"))
        psum_main = ctx.enter_context(tc.tile_pool(name="pmain", bufs=6, space="PSUM"))
        xpool = ctx.enter_context(tc.tile_pool(name="xin", bufs=bufs))
        if obufs:
            opool = ctx.enter_context(tc.tile_pool(name="yout", bufs=obufs))

        # ---- small constant loads (all reads; precede the x-load burst)
        ident_sb = const_pool.tile([128, 128], f32)
        nc.sync.dma_start(ident_sb[:], ident[:])
        e1 = const_pool.tile([128, EMB], f32)
        nc.sync.dma_start(e1[:], emb[0:128, :])
        e2 = const_pool.tile([80, EMB], f32, tag="e2")
        nc.sync.dma_start(e2[:], emb[128:NODE_P, :])
        bias_sb = const_pool.tile([128, F], f32)
        nc.sync.dma_start(bias_sb[:], bias_d[:])
        eye1 = const_pool.tile([128, NODE], f32)
        nc.sync.dma_start(eye1[:], eye[0:128, :])
        eye2 = const_pool.tile([80, NODE], f32, tag="eye2")
        nc.sync.dma_start(eye2[:], eye[128:NODE_P, :])
        w_sb = const_pool.tile([128, F * C], bf16)
        nc.sync.dma_start(w_sb[:], w[:])

        # ---- E^T (unscaled) via PE transpose
        ET = small_pool.tile([128, NODE_P], f32)
        tp1 = psum_prep.tile([128, 128], f32, tag="prep")
        nc.tensor.transpose(tp1[:], e1[:], ident_sb[:])
        nc.vector.tensor_copy(ET[:, 0:128], tp1[:])
        tp2 = psum_prep.tile([128, 80], f32, tag="prep")
        nc.tensor.transpose(tp2[:], e2[:], ident_sb[0:80, 0:80])
        nc.vector.tensor_copy(ET[:, 128:NODE_P], tp2[:])

        # ---- sim = relu(E E^T) per row-tile; d = 1 + softmax diag
        def diag_gate(rows, off, e_tile, eye_tile):
            simp_t = psum_prep.tile([128, NODE_P], f32, tag="prep")
            simp = simp_t[0:rows, :]
            nc.tensor.matmul(simp, ET[:, off : off + rows], ET[:])
            s_t = small_pool.tile([128, NODE_P], f32, tag=f"s{off}")
            s = s_t[0:rows, :]
            nc.vector.tensor_relu(s[:], simp[:])
            m_t = small_pool.tile([128, 1], f32, tag=f"m{off}")
            m = m_t[0:rows, :]
            nc.vector.tensor_reduce(m[:], s[:, 0:NODE], AX.X, ALU.max)
            negm_t = small_pool.tile([128, 1], f32, tag=f"negm{off}")
            negm = negm_t[0:rows, :]
            nc.vector.tensor_scalar_mul(negm[:], m[:], -1.0)
            ex_t = small_pool.tile([128, NODE], f32, tag=f"ex{off}")
            ex = ex_t[0:rows, :]
            nc.scalar.activation(ex[:], s[:, 0:NODE], AF.Exp, bias=negm[:])
            z_t = small_pool.tile([128, 1], f32, tag=f"z{off}")
            z = z_t[0:rows, :]
            nc.vector.tensor_reduce(z[:], ex[:], AX.X, ALU.add)
            msk_t = small_pool.tile([128, NODE], f32, tag=f"msk{off}")
            msk = msk_t[0:rows, :]
            nc.vector.tensor_mul(msk[:], s[:, 0:NODE], eye_tile[:])
            dg_t = small_pool.tile([128, 1], f32, tag=f"dg{off}")
            dg = dg_t[0:rows, :]
            nc.vector.tensor_reduce(dg[:], msk[:], AX.X, ALU.add)
            ed_t = small_pool.tile([128, 1], f32, tag=f"ed{off}")
            ed = ed_t[0:rows, :]
            nc.scalar.activation(ed[:], dg[:], AF.Exp, bias=negm[:])
            rz_t = small_pool.tile([128, 1], f32, tag=f"rz{off}")
            rz = rz_t[0:rows, :]
            nc.vector.reciprocal(rz[:], z[:])
            d_t = small_pool.tile([128, 1], f32, tag=f"d{off}")
            d = d_t[0:rows, :]
            nc.vector.tensor_mul(d[:], ed[:], rz[:])
            nc.vector.tensor_scalar_add(d[:], d[:], 1.0)
            # E' = diag(d) @ E rows
            ep_t = small_pool.tile([128, EMB], f32, tag=f"ep{off}")
            ep = ep_t[0:rows, :]
            nc.vector.tensor_scalar_mul(ep[:], e_tile[:], d[:])
            return ep

        ep1 = diag_gate(128, 0, e1, eye1)
        ep2 = diag_gate(80, 128, e2, eye2)

        # ---- E'^T via PE transpose, narrowed to bf16 for the Ew build
        EpTb = small_pool.tile([128, NODE_P], bf16)
        tq1 = psum_prep.tile([128, 128], f32, tag="prep")
        nc.tensor.transpose(tq1[:], ep1[:], ident_sb[:])
        nc.vector.tensor_copy(EpTb[:, 0:128], tq1[:])
        tq2 = psum_prep.tile([128, 80], f32, tag="prep")
        nc.tensor.transpose(tq2[:], ep2[:], ident_sb[0:80, 0:80])
        nc.vector.tensor_copy(EpTb[:, 128:NODE_P], tq2[:])

        # ---- per-node bias, pair-stacked: ebT2[64*par + f, p] = eb[2p+par, f]
        ebT2 = const_pool.tile([128, PAIRS], f32)
        pe = psum_prep.tile([128, PAIRS], f32, tag="prep")
        nc.tensor.matmul(pe[0:64, :], bias_sb[:], ET[:, 0:NODE_P:2])
        nc.tensor.matmul(pe[64:128, :], bias_sb[:], ET[:, 1:NODE_P:2])
        nc.vector.tensor_copy(ebT2[:], pe[:])

        # ---- stationary weights, block-diagonal per pair (bf16):
        #   Ew[c,       p*128 + f]      = sum_d E'[2p,   d] W[d, c, f]
        #   Ew[64 + c,  p*128 + 64 + f] = sum_d E'[2p+1, d] W[d, c, f]
        # (off-diagonal quadrants stay zero)
        Ew = const_pool.tile([128, 128 * PAIRS], bf16)
        zsrc = small_pool.tile([128, 8 * PAIRS], f32)
        nc.vector.memset(zsrc[:], 0.0)
        Ew3 = Ew[:].rearrange("p (q b) -> p q b", b=128)
        zs3 = zsrc[:].rearrange("p (q b) -> p q b", b=8)
        for s in range(8):
            nc.vector.tensor_copy(Ew3[0:64, :, 64 + 8 * s : 72 + 8 * s], zs3[0:64, :, :])
            nc.vector.tensor_copy(Ew3[64:128, :, 8 * s : 8 * (s + 1)], zs3[64:128, :, :])
        for f in range(F):
            # [128, RH]-shaped so the main pool serves the build too (one
            # shape -> one bank per buffer); 6-way rotation overlaps matmul
            # f+1 with the copies of f. Only cols 0:PAIRS are used.
            pf_t = psum_main.tile([128, RH], f32, name="ps")
            pf = pf_t[:, 0:PAIRS]
            wf = w_sb[:, f * C : (f + 1) * C]  # W[:, f, :] (w is f-major from host)
            nc.tensor.matmul(pf[0:64, :], wf, EpTb[:, 0:NODE_P:2])
            nc.tensor.matmul(pf[64:128, :], wf, EpTb[:, 1:NODE_P:2])
            if f % 2 == 0:
                nc.vector.tensor_copy(Ew[0:64, f :: 128], pf[0:64, :])
                nc.vector.tensor_copy(Ew[64:128, (64 + f) :: 128], pf[64:128, :])
            else:
                nc.scalar.copy(Ew[0:64, f :: 128], pf[0:64, :])
                nc.scalar.copy(Ew[64:128, (64 + f) :: 128], pf[64:128, :])

        # ---- main streaming loop over node pairs, burst-phased on ONE HWDGE
        # ring: the ring's FIFO keeps the load burst and store burst of each
        # phase apart, so HBM traffic stays unidirectional per burst (mixed
        # read/write collapses DMA throughput ~2x, measured)
        st_eng = nc.sync if ring == "sync" else nc.scalar

        def main_loop():
            for k0 in range(0, ng, ph):
                phase = []
                for k in range(k0, min(k0 + ph, ng)):
                    x2 = xpool.tile([128, nb * R], bf16)
                    nc.sync.dma_start(x2[:], xt[k * 128 : (k + 1) * 128, :])
                    phase.append((k, x2))
                for k, x2 in phase:
                    # obufs>0: bias-adds write a separate staging tile so the
                    # x2 tile is freed by the PE's last read (engine sem) and
                    # the next pass's load never waits on a store completion
                    out = opool.tile([128, nb * R], bf16, name="out") if obufs else x2
                    for j in range(nb):
                        p = k * nb + j
                        ew_p = Ew[:, p * 128 : (p + 1) * 128]
                        for h in range(2):
                            ps = psum_main.tile([128, RH], f32)
                            cols = slice(j * R + h * RH, j * R + (h + 1) * RH)
                            nc.tensor.matmul(ps[:], ew_p, x2[:, cols])
                            # alternate DVE / ACT so neither engine bottlenecks
                            if h == 0:
                                nc.vector.tensor_scalar_add(
                                    out[:, cols], ps[:], ebT2[:, p : p + 1]
                                )
                            else:
                                nc.scalar.add(out[:, cols], ps[:], ebT2[:, p : p + 1])
                    st_eng.dma_start(yt[k * 128 : (k + 1) * 128, :], out[:])

        if repeat == 1:
            main_loop()
        else:
            # hardware loop: one NEFF execution = `repeat` full streaming passes
            # (benchmarking only; kernel() uses repeat=1)
            with tc.For_i(0, repeat, 1):
                main_loop()

    nc.compile()
    return nc


def _get_nc(
    repeat=1,
    bufs=None,
    ph=None,
    nb=NB,
    ring="sync",
    mode="normal",
    ncores=N_CORES,
    obufs=0,
):
    key = f"nc{repeat}_{bufs}_{ph}_{nb}_{ring}_{mode}_{ncores}_{obufs}"
    if key not in _CACHE:
        _CACHE[key] = _build(repeat, bufs, ph, nb, ring, mode, ncores, obufs)
    return _CACHE[key]


def host_in_maps(x, node_embedding, weights, bias, nb=NB):
    """Shard + permute full inputs into per-core input maps (bf16 wire)."""
    import ml_dtypes

    bf = ml_dtypes.bfloat16
    ng = PAIRS // nb

    emb_p = np.zeros((NODE_P, EMB), np.float32)
    emb_p[:NODE] = node_embedding
    w2 = np.ascontiguousarray(
        np.asarray(weights).transpose(0, 2, 1).reshape(EMB, F * C)
    ).astype(bf)
    bias_f = np.ascontiguousarray(bias, np.float32)
    eye_np = np.eye(NODE_P, NODE, dtype=np.float32)
    ident_np = np.eye(128, dtype=np.float32)

    in_maps = []
    for i in range(N_CORES):
        xi = x[B_SH * i : B_SH * (i + 1)]  # [2, T, NODE, C]
        xt = np.zeros((NODE_P, C, R), bf)
        xt[:NODE] = np.asarray(xi).transpose(2, 3, 0, 1).reshape(NODE, C, R).astype(bf)
        # group nb pairs: [G, nb, 2, C, R] -> [G, (2, C)=128, nb, R] so each
        # partition line is one contiguous nb*R*2-byte DMA descriptor
        xt_g = np.ascontiguousarray(
            xt.reshape(ng, nb, 2, C, R).transpose(0, 2, 3, 1, 4)
        ).reshape(ng * 128, nb * R)
        in_maps.append(
            {
                "xt": xt_g,
                "emb": emb_p,
                "w": w2,
                "bias": bias_f,
                "eye": eye_np,
                "ident": ident_np,
            }
        )
    return in_maps


def host_out(results, nb=NB):
    """Un-permute per-core yt shards back to the full [B, T, N, F] f32 output."""
    ng = PAIRS // nb
    out = np.empty((B, T, NODE, F), np.float32)
    for i in range(N_CORES):
        ytr = results[i]["yt"].reshape(ng, 2, F, nb, B_SH, T).astype(np.float32)
        y_local = ytr.transpose(4, 5, 0, 3, 1, 2).reshape(B_SH, T, NODE_P, F)
        out[B_SH * i : B_SH * (i + 1)] = y_local[:, :, :NODE, :]
    return out


def _spot_reference(x, node_embedding, weights, bias, n_rows=2):
    """Reference y[0, :n_rows] in f32 numpy — cheap guard against transient
    garbage from a wedged device (observed once: whole-output corruption)."""
    E = np.asarray(node_embedding, np.float64)
    sim = np.maximum(E @ E.T, 0.0)
    ex = np.exp(sim - sim.max(axis=1, keepdims=True))
    d = 1.0 + np.diag(ex / ex.sum(axis=1, keepdims=True))
    Ew = np.einsum(
        "nd,dcf->ncf", d[:, None] * E, np.asarray(weights, np.float64)
    )
    eb = E @ np.asarray(bias, np.float64)
    xs = np.asarray(x[0, :n_rows], np.float64)  # [n_rows, NODE, C]
    return (np.einsum("tnc,ncf->tnf", xs, Ew) + eb).astype(np.float32)


def kernel(x, node_embedding, weights, bias):
    from concourse.bass_utils import run_bass_kernel_spmd

    nc = _get_nc()
    in_maps = host_in_maps(x, node_embedding, weights, bias)
    y_spot = _spot_reference(x, node_embedding, weights, bias)
    scale = np.abs(y_spot).max() + 1e-30
    for attempt in range(2):
        res = run_bass_kernel_spmd(nc, in_maps, core_ids=list(range(N_CORES)))
        out = host_out(res.results)
        err = np.abs(out[0, : y_spot.shape[0]] - y_spot).max() / scale
        if err < 0.05:  # bf16 wire error is ~5e-3; garbage is >>1
            break
    return out
